# revision 11
# baseline (speedup 1.0000x reference)
"""Self-contained Trainium2 Bass kernel for a 4-layer GCN (nn_GCN4).

Strategy (8 NeuronCores, SPMD):
- Nodes are placed (host-side) into 8 slabs of 6272 padded slots (49 blocks
  of 128), balanced so every 128-dst block has a near-equal number of
  incoming edges from each half of the table (the halves keep dma_gather
  indices within int16 range).
- Per layer: transform locally (dense matmul on the core's 6272 nodes),
  AllGather the bf16 transformed table, then aggregate: dma_gather source
  rows per edge (round-robined over 4 SWDGE queues — the gather is
  descriptor-rate-bound, ~8ns/row/queue), build a one-hot scatter matrix on
  the vector engine (iota == rel), and scatter-add via TensorE matmul into
  PSUM.
- Self-loop messages are not gathered: each block's own transformed tile is
  kept resident in SBUF and injected with one identity matmul per block.
- Symmetric normalization deg^-1/2 factors are folded into the table
  (pre-scale) and the activation epilogues (post-scale); the bias is
  injected as a rank-1 "bias wave" matmul so ReLU commutes with the
  deferred scale.
- The one-hot S matrices are layer-invariant; the first SCACHE blocks' S
  tiles are built once and reused across all four layers.
"""
import math
import numpy as np

import concourse.bass as bass
import concourse.bacc as bacc
import concourse.mybir as mybir
import concourse.tile as tile
from concourse import bass_utils

BF = mybir.dt.np(mybir.dt.bfloat16)


class Cfg:
    def __init__(self, N, E, R, NB, group=4, scache=4, seed=0):
        self.N = N          # real nodes
        self.E = E          # directed edges (before self loops)
        self.R = R          # cores
        self.NB = NB        # 128-row blocks per core
        self.SLOTS = NB * 128
        self.NPAD = R * self.SLOTS
        self.HALF = self.NPAD // 2
        self.NBG = R * NB   # global blocks
        self.GROUP = group  # blocks per gather group
        self.SCACHE = scache
        self.seed = seed
        self.F_IN = 128
        self.H1 = 256
        self.H2 = 128
        self.H3 = 64
        self.C = 40
        assert self.N <= self.NPAD and (self.N + 1) // 2 <= self.HALF


REAL = Cfg(N=50000, E=800000, R=8, NB=49)


# ----------------------------------------------------------------------------
# Host preprocessing
# ----------------------------------------------------------------------------

def _pack_half(nodes, dlo, dhi, nblocks):
    """Greedy 2D bin-packing of nodes into blocks of 128 slots, balancing
    both lo- and hi- incoming edge counts per block."""
    order = np.argsort(-(dlo + dhi), kind="stable")
    nodes = nodes[order]
    dlo = dlo[order].astype(np.float64)
    dhi = dhi[order].astype(np.float64)
    losum = np.zeros(nblocks)
    hisum = np.zeros(nblocks)
    cnt = np.zeros(nblocks, np.int64)
    Lt = max(dlo.sum() / nblocks, 1.0)
    Ht = max(dhi.sum() / nblocks, 1.0)
    blk = np.empty(len(nodes), np.int64)
    for i in range(len(nodes)):
        score = np.maximum((losum + dlo[i]) / Lt, (hisum + dhi[i]) / Ht)
        score[cnt >= 128] = np.inf
        b = int(np.argmin(score))
        blk[i] = b
        losum[b] += dlo[i]
        hisum[b] += dhi[i]
        cnt[b] += 1
    return nodes, blk


def preprocess(cfg, x, edge_index, W1, b1, W2, b2, W3, b3, W4, b4):
    N, R, NB = cfg.N, cfg.R, cfg.NB
    SLOTS, NPAD, HALF, NBG = cfg.SLOTS, cfg.NPAD, cfg.HALF, cfg.NBG

    src = np.asarray(edge_index[0], np.int64)
    dst = np.asarray(edge_index[1], np.int64)
    loops = np.arange(N, dtype=np.int64)
    # degree includes self-loops (PyG default), but loops are NOT gathered —
    # they are injected on-chip from the resident own-block tiles.
    deg = np.bincount(np.concatenate([dst, loops]), minlength=N).astype(np.float64)
    dinv = 1.0 / np.sqrt(deg)          # deg >= 1 (self loop)
    rdeg = np.sqrt(deg)

    src_all = src
    dst_all = dst
    M = len(src_all)

    rng = np.random.default_rng(cfg.seed)
    perm = rng.permutation(N)
    N_lo = N // 2
    is_lo_node = np.zeros(N, bool)
    is_lo_node[perm[:N_lo]] = True

    deg_lo = np.bincount(dst_all[is_lo_node[src_all]], minlength=N).astype(np.int64)
    deg_hi = np.bincount(dst_all, minlength=N).astype(np.int64) - deg_lo

    pos = np.full(N, -1, np.int64)
    half_nb = NBG // 2
    for half, nodeset in ((0, perm[:N_lo]), (1, perm[N_lo:])):
        nodes, blk = _pack_half(nodeset, deg_lo[nodeset], deg_hi[nodeset], half_nb)
        # slot order within block = assignment order
        o = np.argsort(blk, kind="stable")
        nodes_s = nodes[o]
        blk_s = blk[o]
        slot = np.arange(len(nodes_s)) - np.searchsorted(blk_s, blk_s)
        pos[nodes_s] = half * HALF + blk_s * 128 + slot

    inv_pos = np.full(NPAD, -1, np.int64)
    inv_pos[pos] = np.arange(N)

    # Edge arrays
    p_src = pos[src_all]
    p_dst = pos[dst_all]
    bg = p_dst >> 7
    rel = (p_dst & 127).astype(np.float32)
    sec = (p_src >= HALF).astype(np.int64)
    gidx = (p_src - sec * HALF).astype(np.int64)
    assert gidx.max() < HALF <= 32768

    cnt2 = np.bincount(bg * 2 + sec, minlength=NBG * 2)
    TLO = int(math.ceil(cnt2[0::2].max() / 128))
    THI = int(math.ceil(cnt2[1::2].max() / 128))
    T = TLO + THI

    A_idx = np.zeros((NBG, T, 128), np.int16)
    A_rel = np.full((NBG, T, 128), -1.0, np.float32)

    key = bg * 2 + sec
    order = np.argsort(key, kind="stable")
    key_s = key[order]
    starts = np.concatenate([[0], np.cumsum(np.bincount(key_s, minlength=NBG * 2))])
    rank = np.arange(M) - starts[key_s]
    t_s = rank // 128 + np.where(sec[order] == 1, TLO, 0)
    p_s = rank % 128
    A_idx[bg[order], t_s, p_s] = gidx[order].astype(np.int16)
    A_rel[bg[order], t_s, p_s] = rel[order]

    # Per-position node attributes
    dinv_pos = np.zeros(NPAD, np.float64)
    rdeg_pos = np.zeros(NPAD, np.float64)
    dinv_pos[pos] = dinv
    rdeg_pos[pos] = rdeg

    xp = np.zeros((NPAD, cfg.F_IN), np.float32)
    xp[pos] = np.asarray(x, np.float32) * dinv[:, None]
    xp = xp.astype(BF)

    # Per-core inputs
    def wrap(a):
        # a: [nblk, Tsec, 128] -> wrapped [128, nblk*Tsec*8] int16
        flat = a.reshape(-1)
        w = flat.reshape(-1, 16).T            # [16, n/16]
        return np.tile(w, (8, 1)).astype(np.int16)

    in_maps = []
    for r in range(R):
        bl = slice(r * NB, (r + 1) * NB)
        sl = slice(r * SLOTS, (r + 1) * SLOTS)
        grel = A_rel[bl].transpose(2, 0, 1).reshape(128, NB * T)  # [p, b*T+t]
        ownx = xp[sl].reshape(NB, 128, cfg.F_IN).transpose(1, 0, 2)
        m = {
            "xp": xp,
            "ownx": np.ascontiguousarray(ownx),
            "ident": np.eye(128, dtype=np.float32).astype(BF),
            "idx_lo": wrap(A_idx[bl, :TLO, :]),
            "idx_hi": wrap(A_idx[bl, TLO:, :]),
            "grel": grel.astype(BF),
            "iota": np.tile(np.arange(128, dtype=np.float32), (128, 1)).astype(BF),
            "dinvp": dinv_pos[sl].reshape(NB, 128).T.astype(np.float32).copy(),
            "dinv2p": (dinv_pos[sl] ** 2).reshape(NB, 128).T.astype(np.float32).copy(),
            "rdegb": rdeg_pos[sl].reshape(1, SLOTS).astype(BF),
            "w1": np.asarray(W1, np.float32).astype(BF),
            "w2": np.asarray(W2, np.float32).reshape(2, 128, cfg.H2)
                    .transpose(1, 0, 2).astype(BF),
            "w3": np.asarray(W3, np.float32).astype(BF),
            "w4": np.asarray(W4, np.float32).astype(BF),
            "b1": np.asarray(b1, np.float32).reshape(1, -1).astype(BF),
            "b2": np.asarray(b2, np.float32).reshape(1, -1).astype(BF),
            "b3": np.asarray(b3, np.float32).reshape(1, -1).astype(BF),
            "b4": np.asarray(b4, np.float32).reshape(1, -1).astype(BF),
        }
        in_maps.append(m)

    struct = (TLO, THI)
    return in_maps, struct, inv_pos


# ----------------------------------------------------------------------------
# Bass program
# ----------------------------------------------------------------------------

def build(cfg, TLO, THI, collectives=True):
    NB, SLOTS, NPAD, HALF = cfg.NB, cfg.SLOTS, cfg.NPAD, cfg.HALF
    T = TLO + THI
    SC = cfg.SCACHE
    bf16 = mybir.dt.bfloat16
    f32 = mybir.dt.float32
    RELU = mybir.ActivationFunctionType.Relu
    COPY = mybir.ActivationFunctionType.Copy

    groups = []
    b0 = 0
    while b0 < NB:
        nbk = min(cfg.GROUP, NB - b0)
        groups.append((b0, nbk))
        b0 += nbk

    nc = bacc.Bacc("TRN2", target_bir_lowering=False, debug=False,
                   num_devices=cfg.R, num_swdge_queues=4)
    rg = [list(range(cfg.R))]

    # I/O
    xp_d = nc.dram_tensor("xp", [NPAD, cfg.F_IN], bf16, kind="ExternalInput")
    ownx_d = nc.dram_tensor("ownx", [128, NB, cfg.F_IN], bf16, kind="ExternalInput")
    ident_d = nc.dram_tensor("ident", [128, 128], bf16, kind="ExternalInput")
    idx_lo_d = nc.dram_tensor("idx_lo", [128, NB * TLO * 8], mybir.dt.int16, kind="ExternalInput")
    idx_hi_d = nc.dram_tensor("idx_hi", [128, NB * THI * 8], mybir.dt.int16, kind="ExternalInput")
    grel_d = nc.dram_tensor("grel", [128, NB * T], bf16, kind="ExternalInput")
    iota_d = nc.dram_tensor("iota", [128, 128], bf16, kind="ExternalInput")
    dinvp_d = nc.dram_tensor("dinvp", [128, NB], f32, kind="ExternalInput")
    dinv2p_d = nc.dram_tensor("dinv2p", [128, NB], f32, kind="ExternalInput")
    rdegb_d = nc.dram_tensor("rdegb", [1, SLOTS], bf16, kind="ExternalInput")
    w1_d = nc.dram_tensor("w1", [128, cfg.H1], bf16, kind="ExternalInput")
    w2_d = nc.dram_tensor("w2", [128, 2, cfg.H2], bf16, kind="ExternalInput")
    w3_d = nc.dram_tensor("w3", [cfg.H2, cfg.H3], bf16, kind="ExternalInput")
    w4_d = nc.dram_tensor("w4", [cfg.H3, cfg.C], bf16, kind="ExternalInput")
    b1_d = nc.dram_tensor("b1", [1, cfg.H1], bf16, kind="ExternalInput")
    b2_d = nc.dram_tensor("b2", [1, cfg.H2], bf16, kind="ExternalInput")
    b3_d = nc.dram_tensor("b3", [1, cfg.H3], bf16, kind="ExternalInput")
    b4_d = nc.dram_tensor("b4", [1, cfg.C], bf16, kind="ExternalInput")
    out_d = nc.dram_tensor("out", [SLOTS, cfg.C], f32, kind="ExternalOutput")

    shared = "Shared" if cfg.R > 4 else "Local"
    ag2in = nc.dram_tensor("ag2in", [SLOTS, 128], bf16, kind="Internal")
    ag2out = nc.dram_tensor("ag2out", [NPAD, 128], bf16, kind="Internal", addr_space=shared)
    ag3in = nc.dram_tensor("ag3in", [SLOTS, 128], bf16, kind="Internal")
    ag3out = nc.dram_tensor("ag3out", [NPAD, 128], bf16, kind="Internal", addr_space=shared)
    ag4in = nc.dram_tensor("ag4in", [SLOTS, 128], bf16, kind="Internal")
    ag4out = nc.dram_tensor("ag4out", [NPAD, 128], bf16, kind="Internal", addr_space=shared)

    qctr = [0]

    def nextq():
        q = qctr[0] % 4
        qctr[0] += 1
        return q

    with tile.TileContext(nc) as tc:
        with (
            tc.tile_pool(name="res", bufs=1) as res,          # resident
            tc.tile_pool(name="gat", bufs=3) as gat,
            tc.tile_pool(name="sp", bufs=3) as sp,
            tc.tile_pool(name="epi", bufs=3) as epi,
            tc.tile_pool(name="aps", bufs=4, space="PSUM") as aps,
            tc.tile_pool(name="tps", bufs=2, space="PSUM") as tps,
        ):
            # residents
            grel_t = res.tile([128, NB * T], bf16)
            ilo_t = res.tile([128, NB * TLO * 8], mybir.dt.int16)
            ihi_t = res.tile([128, NB * THI * 8], mybir.dt.int16)
            iota_t = res.tile([128, 128], bf16)
            ident_t = res.tile([128, 128], bf16)
            ownx_t = res.tile([128, NB, cfg.F_IN], bf16)
            dinvp_t = res.tile([128, NB], f32)
            dinv2p_t = res.tile([128, NB], f32)
            rdegb_t = res.tile([1, SLOTS], bf16)
            w1_t = res.tile([128, cfg.H1], bf16)
            w2_t = res.tile([128, 2, cfg.H2], bf16)
            w3_t = res.tile([cfg.H2, cfg.H3], bf16)
            w4_t = res.tile([cfg.H3, cfg.C], bf16)
            b1_t = res.tile([1, cfg.H1], bf16)
            b2_t = res.tile([1, cfg.H2], bf16)
            b3_t = res.tile([1, cfg.H3], bf16)
            b4_t = res.tile([1, cfg.C], bf16)
            for t, d in ((grel_t, grel_d), (ilo_t, idx_lo_d), (ihi_t, idx_hi_d),
                         (iota_t, iota_d), (ident_t, ident_d),
                         (ownx_t, ownx_d), (dinvp_t, dinvp_d),
                         (dinv2p_t, dinv2p_d), (rdegb_t, rdegb_d),
                         (w1_t, w1_d), (w2_t, w2_d), (w3_t, w3_d), (w4_t, w4_d),
                         (b1_t, b1_d), (b2_t, b2_d), (b3_t, b3_d), (b4_t, b4_d)):
                nc.sync.dma_start(t[:], d[:])

            agg1T = res.tile([128, SLOTS], bf16)   # L1 raw aggregate, feature-major
            h1T0 = res.tile([128, SLOTS], bf16)    # relu(agg1T@W1 + bias), j-tile 0
            h1T1 = res.tile([128, SLOTS], bf16)
            h2T = res.tile([128, SLOTS], bf16)
            h3T = res.tile([cfg.H3, SLOTS], bf16)
            own2 = res.tile([128, NB, 128], bf16)  # own transformed tiles (self loops)
            own3 = res.tile([128, NB, cfg.H3], bf16)
            own4 = res.tile([128, NB, cfg.C], bf16)
            s_cache = [res.tile([128, T, 128], bf16, name=f"s_cache{i}")
                       for i in range(SC)]

            iota_b = iota_t[:].unsqueeze(1).broadcast_to([128, T, 128])

            def make_S(b, first):
                """One-hot scatter matrix for block b; cached for b < SCACHE."""
                if b < SC:
                    S = s_cache[b]
                    if not first:
                        return S
                else:
                    S = sp.tile([128, T, 128], bf16, tag="S")
                nc.vector.tensor_tensor(
                    S[:],
                    iota_b,
                    grel_t[:, b * T:(b + 1) * T].unsqueeze(2).broadcast_to([128, T, 128]),
                    mybir.AluOpType.is_equal)
                return S

            def aggregate(table_d, FW, out_cb, bias_wave, self_ap, first=False):
                """Generic aggregation layer.
                table_d: DRAM table [NPAD, 128] bf16 (gather source)
                FW: lhsT feature width (cols of gathered tile used)
                out_cb(b, psum): epilogue for block b
                bias_wave(b, psum): starts accumulation; return True if it
                  issued a start matmul.
                self_ap(b): lhsT [128, FW] for the self-loop identity matmul."""
                lo_ap = table_d[0:HALF, :]
                hi_ap = table_d[HALF:NPAD, :]
                for (g0, nbk) in groups:
                    glo = gat.tile([128, cfg.GROUP * TLO, 128], bf16, tag="glo")
                    ghi = gat.tile([128, cfg.GROUP * THI, 128], bf16, tag="ghi")
                    nlo = nbk * TLO * 128
                    nhi = nbk * THI * 128
                    nc.gpsimd.dma_gather(
                        glo[:, :nbk * TLO, :], lo_ap,
                        ilo_t[:, g0 * TLO * 8:(g0 + nbk) * TLO * 8],
                        num_idxs=nlo, num_idxs_reg=nlo, elem_size=128, single_packet=False,
                        queue_num=nextq())
                    nc.gpsimd.dma_gather(
                        ghi[:, :nbk * THI, :], hi_ap,
                        ihi_t[:, g0 * THI * 8:(g0 + nbk) * THI * 8],
                        num_idxs=nhi, num_idxs_reg=nhi, elem_size=128, single_packet=False,
                        queue_num=nextq())
                    for k in range(nbk):
                        b = g0 + k
                        S = make_S(b, first)
                        psum = aps.tile([FW, 128], f32, tag="agg")
                        started = bias_wave(b, psum)
                        nc.tensor.matmul(psum[:], self_ap(b), ident_t[:],
                                         start=not started, stop=False)
                        for t in range(T):
                            if t < TLO:
                                g_ap = glo[:, k * TLO + t, :FW]
                            else:
                                g_ap = ghi[:, k * THI + (t - TLO), :FW]
                            nc.tensor.matmul(psum[:], g_ap, S[:, t, :],
                                             start=False, stop=(t == T - 1))
                        out_cb(b, psum)

            # ---------------- L1 aggregation (table = xp) ----------------
            def l1_out(b, psum):
                nc.vector.tensor_copy(agg1T[:, b * 128:(b + 1) * 128], psum[:])

            aggregate(xp_d, 128, l1_out, lambda b, p: False,
                      lambda b: ownx_t[:, b, :], first=True)

            # ---------------- L1 transform -> h1T ----------------
            v0 = 0
            while v0 < SLOTS:
                vsz = min(512, SLOTS - v0)
                for j in range(2):
                    pt = tps.tile([128, 512], f32, tag="tps")
                    nc.tensor.matmul(pt[:, :vsz], w1_t[:, j * 128:(j + 1) * 128],
                                     agg1T[:, v0:v0 + vsz], start=True, stop=False)
                    nc.tensor.matmul(pt[:, :vsz], b1_t[0:1, j * 128:(j + 1) * 128],
                                     rdegb_t[0:1, v0:v0 + vsz], start=False, stop=True)
                    h = h1T0 if j == 0 else h1T1
                    nc.scalar.activation(h[:, v0:v0 + vsz], pt[:, :vsz], RELU)
                v0 += vsz

            # ---------------- L2 transform -> ag2in; AllGather ----------------
            for b in range(NB):
                bs = slice(b * 128, (b + 1) * 128)
                pt = tps.tile([128, 512], f32, tag="tps")
                nc.tensor.matmul(pt[:, :128], h1T0[:, bs], w2_t[:, 0, :], start=True, stop=False)
                nc.tensor.matmul(pt[:, :128], h1T1[:, bs], w2_t[:, 1, :], start=False, stop=True)
                nc.scalar.activation(own2[:, b, :], pt[:, :128], COPY, scale=dinv2p_t[:, b:b + 1])
                nc.sync.dma_start(ag2in[bs, :], own2[:, b, :])
            if collectives:
                nc.gpsimd.collective_compute("AllGather", mybir.AluOpType.bypass,
                                             replica_groups=rg, ins=[ag2in[:]], outs=[ag2out[:]])
            else:
                nc.sync.dma_start(ag2out[0:SLOTS, :], ag2in[:])

            # ---------------- L2 aggregation -> h2T ----------------
            def l2_bias(b, psum):
                nc.tensor.matmul(psum[:], b2_t[:], rdegb_t[0:1, b * 128:(b + 1) * 128],
                                 start=True, stop=False)
                return True

            def l2_out(b, psum):
                nc.scalar.activation(h2T[:, b * 128:(b + 1) * 128], psum[:], RELU)

            aggregate(ag2out, 128, l2_out, l2_bias, lambda b: own2[:, b, :])

            # ---------------- L3 transform -> ag3in; AllGather ----------------
            for b in range(NB):
                bs = slice(b * 128, (b + 1) * 128)
                pt = tps.tile([128, 512], f32, tag="tps")
                nc.tensor.matmul(pt[:, :cfg.H3], h2T[:, bs], w3_t[:], start=True, stop=True)
                nc.scalar.activation(own3[:, b, :], pt[:, :cfg.H3], COPY, scale=dinv2p_t[:, b:b + 1])
                nc.sync.dma_start(ag3in[bs, 0:cfg.H3], own3[:, b, :])
            if collectives:
                nc.gpsimd.collective_compute("AllGather", mybir.AluOpType.bypass,
                                             replica_groups=rg, ins=[ag3in[:]], outs=[ag3out[:]])
            else:
                nc.sync.dma_start(ag3out[0:SLOTS, :], ag3in[:])

            # ---------------- L3 aggregation -> h3T ----------------
            def l3_bias(b, psum):
                nc.tensor.matmul(psum[:], b3_t[:], rdegb_t[0:1, b * 128:(b + 1) * 128],
                                 start=True, stop=False)
                return True

            def l3_out(b, psum):
                nc.scalar.activation(h3T[:, b * 128:(b + 1) * 128], psum[:], RELU)

            aggregate(ag3out, cfg.H3, l3_out, l3_bias, lambda b: own3[:, b, :])

            # ---------------- L4 transform -> ag4in; AllGather ----------------
            for b in range(NB):
                bs = slice(b * 128, (b + 1) * 128)
                pt = tps.tile([128, 512], f32, tag="tps")
                nc.tensor.matmul(pt[:, :cfg.C], h3T[:, bs], w4_t[:], start=True, stop=True)
                nc.scalar.activation(own4[:, b, :], pt[:, :cfg.C], COPY, scale=dinv2p_t[:, b:b + 1])
                nc.sync.dma_start(ag4in[bs, 0:cfg.C], own4[:, b, :])
            if collectives:
                nc.gpsimd.collective_compute("AllGather", mybir.AluOpType.bypass,
                                             replica_groups=rg, ins=[ag4in[:]], outs=[ag4out[:]])
            else:
                nc.sync.dma_start(ag4out[0:SLOTS, :], ag4in[:])

            # ---------------- L4 aggregation (node-major) -> out ----------------
            def l4_bias(b, psum):
                nc.tensor.matmul(psum[:], rdegb_t[0:1, b * 128:(b + 1) * 128], b4_t[:],
                                 start=True, stop=False)
                return True

            def l4_out(b, psum):
                o = epi.tile([128, cfg.C], f32, tag="o4")
                nc.scalar.activation(o[:], psum[:], COPY, scale=dinvp_t[:, b:b + 1])
                nc.sync.dma_start(out_d[b * 128:(b + 1) * 128, :], o[:])

            # node-major: lhsT = S slice, rhs = gathered cols 0:C
            lo_ap = ag4out[0:HALF, :]
            hi_ap = ag4out[HALF:NPAD, :]
            for (g0, nbk) in groups:
                glo = gat.tile([128, cfg.GROUP * TLO, 128], bf16, tag="glo")
                ghi = gat.tile([128, cfg.GROUP * THI, 128], bf16, tag="ghi")
                nlo = nbk * TLO * 128
                nhi = nbk * THI * 128
                nc.gpsimd.dma_gather(glo[:, :nbk * TLO, :], lo_ap,
                                     ilo_t[:, g0 * TLO * 8:(g0 + nbk) * TLO * 8],
                                     num_idxs=nlo, num_idxs_reg=nlo, elem_size=128, single_packet=False,
                                     queue_num=nextq())
                nc.gpsimd.dma_gather(ghi[:, :nbk * THI, :], hi_ap,
                                     ihi_t[:, g0 * THI * 8:(g0 + nbk) * THI * 8],
                                     num_idxs=nhi, num_idxs_reg=nhi, elem_size=128, single_packet=False,
                                     queue_num=nextq())
                for k in range(nbk):
                    b = g0 + k
                    S = make_S(b, False)
                    psum = aps.tile([128, cfg.C], f32, tag="agg")
                    l4_bias(b, psum)
                    nc.tensor.matmul(psum[:], ident_t[:], own4[:, b, :],
                                     start=False, stop=False)
                    for t in range(T):
                        if t < TLO:
                            g_ap = glo[:, k * TLO + t, 0:cfg.C]
                        else:
                            g_ap = ghi[:, k * THI + (t - TLO), 0:cfg.C]
                        nc.tensor.matmul(psum[:], S[:, t, :], g_ap,
                                         start=False, stop=(t == T - 1))
                    l4_out(b, psum)

    nc.compile()
    return nc


def build_nocoll(cfg, TLO, THI):
    return build(cfg, TLO, THI, collectives=False)


# ----------------------------------------------------------------------------
# Driver
# ----------------------------------------------------------------------------

_CACHE = {}


def run(cfg, inputs, trace=False):
    in_maps, struct, inv_pos = preprocess(cfg, **inputs)
    key = (cfg.N, cfg.E, cfg.R, cfg.NB) + struct
    if key not in _CACHE:
        _CACHE[key] = build(cfg, *struct)
    nc = _CACHE[key]
    res = bass_utils.run_bass_kernel_spmd(
        nc, in_maps, core_ids=list(range(cfg.R)), trace=trace)
    outs = [res.results[r]["out"] for r in range(cfg.R)]
    full = np.concatenate(outs, axis=0)          # [NPAD, C]
    out = np.empty((cfg.N, cfg.C), np.float32)
    valid = inv_pos >= 0
    out[inv_pos[valid]] = full[valid]
    return out, res


def kernel(**inputs):
    out, _ = run(REAL, inputs)
    return out
